# revision 2
# baseline (speedup 1.0000x reference)
"""Trainium2 Bass kernel v4 (self-contained).

v2 math (causal slotting, transposed scores, bf16 operands) + v3 pipeline,
restructured to keep whid streamed once per layer:

  layer l body:
    hqT(l) | attn 0 | attn 1 | LN2A+upA | attn 2 | attn 3 | upB
    | hid (full, single whid stream) | downA | LN1+hov+fire(l+1, tiles 0,1)
    | downB | LN1+hov+fire(l+1, tiles 2,3)
  last layer fires the 8-core Shared xT gathers after each down half;
  unembed drains them.

All weight DRAM layouts are pre-swizzled so every SBUF tile load is >=1KB
contiguous per partition (DMA descriptor efficiency).
"""

import numpy as np
from contextlib import ExitStack

import concourse.bass as bass
import concourse.bacc as bacc
import concourse.tile as tile
from concourse import mybir
from concourse.bass_utils import run_bass_kernel_spmd
from concourse.masks import make_identity

F32 = mybir.dt.float32
BF16 = mybir.dt.bfloat16
AF = mybir.ActivationFunctionType
OP = mybir.AluOpType

P = 128
D = 1024
H = 4096
L = 4
V = 32000
B = 2
S = 2048
T = B * S
NCORES = 8
TPC = T // NCORES    # 512
VSH = V // NCORES    # 4000
KD = D // P          # 8
KH = H // P          # 32
NT = TPC // P        # 4
NVC = 8
VC = VSH // NVC      # 500
LRELU = 0.01
SCALE = 1.0 / float(np.sqrt(D))
MASKV = -1e9
GROUPS_BATCH = [[0, 1, 2, 3], [4, 5, 6, 7]]
GROUP_ALL = [[0, 1, 2, 3, 4, 5, 6, 7]]


def build_program():
    nc = bacc.Bacc(None, num_devices=NCORES)

    x0 = nc.dram_tensor("x0", [TPC, D], F32, kind="ExternalInput")
    maskT = nc.dram_tensor("maskT", [NT, 4, P, P], F32, kind="ExternalInput")
    # pre-swizzled weights: inner dims contiguous per partition
    qkt = nc.dram_tensor("qkt", [L, KD, P, KD, P], BF16, kind="ExternalInput")
    ovt = nc.dram_tensor("ovt", [L, KD, P, D], BF16, kind="ExternalInput")
    wup = nc.dram_tensor("wup", [L, KH, P, KD, P], BF16, kind="ExternalInput")
    whid = nc.dram_tensor("whid", [L, KH, P, KH, P], BF16, kind="ExternalInput")
    wdn = nc.dram_tensor("wdn", [L, KH, P, D], BF16, kind="ExternalInput")
    bup_all = nc.dram_tensor("bup", [L, H], F32, kind="ExternalInput")
    bhid_all = nc.dram_tensor("bhid", [L, H], F32, kind="ExternalInput")
    embT = nc.dram_tensor("embT", [D, VSH], BF16, kind="ExternalInput")
    logits = nc.dram_tensor("logits", [T, VSH], BF16, kind="ExternalOutput")

    with tile.TileContext(nc) as tc, ExitStack() as ctx:
        const = ctx.enter_context(tc.tile_pool(name="const", bufs=1))
        state = ctx.enter_context(tc.tile_pool(name="state", bufs=1))
        actp = ctx.enter_context(tc.tile_pool(name="actp", bufs=1))
        bigp = ctx.enter_context(tc.tile_pool(name="bigp", bufs=1))
        small = ctx.enter_context(tc.tile_pool(name="small", bufs=4))
        stream = ctx.enter_context(tc.tile_pool(name="stream", bufs=2))
        st3 = ctx.enter_context(tc.tile_pool(name="st3", bufs=3))
        ps = ctx.enter_context(tc.tile_pool(name="ps", bufs=1, space="PSUM"))
        dram = ctx.enter_context(tc.tile_pool(name="dram", bufs=2, space="DRAM"))

        identB = const.tile([P, P], BF16)
        make_identity(nc, identB[:])
        eps_t = const.tile([P, 1], F32)
        nc.vector.memset(eps_t, 1e-5)
        ones_t = const.tile([P, 1], BF16)
        nc.vector.memset(ones_t, 1.0)
        mask_sb = const.tile([P, NT, 4, P], F32)
        nc.sync.dma_start(out=mask_sb,
                          in_=maskT.rearrange("j t k q -> k j t q"))

        x_sb = state.tile([P, NT, D], F32)
        nc.sync.dma_start(out=x_sb,
                          in_=x0.rearrange("(tt p) d -> p tt d", p=P))

        def transpose_to(src_block, dst_ap):
            tp = ps.tile([P, P], BF16, name="trps", tag="tr", bufs=2)
            nc.tensor.transpose(out=tp[:], in_=src_block, identity=identB[:])
            nc.any.tensor_copy(out=dst_ap, in_=tp[:])

        def mm_tile(cols=512):
            return ps.tile([P, cols], F32, name="mm", tag="mm", bufs=5,
                           padded_shape=[P, 512])

        def ln_tile(x_ap, out_ap):
            stats = small.tile([P, 2, nc.vector.BN_STATS_DIM], F32,
                               name="lnst", tag="lnst")
            xv = x_ap.rearrange("p (s f) -> p s f", s=2)
            for s in range(2):
                nc.vector.bn_stats(out=stats[:, s, :], in_=xv[:, s, :])
            mv = small.tile([P, nc.vector.BN_AGGR_DIM], F32, name="lnmv",
                            tag="lnmv")
            nc.vector.bn_aggr(out=mv[:], in_=stats[:])
            rstd = small.tile([P, 1], F32, name="lnrs", tag="lnrs")
            nc.scalar.activation(out=rstd[:], in_=mv[:, 1:2], func=AF.Sqrt,
                                 bias=eps_t[:])
            nc.vector.reciprocal(out=rstd[:], in_=rstd[:])
            nc.vector.tensor_scalar(out=out_ap, in0=x_ap,
                                    scalar1=mv[:, 0:1], scalar2=rstd[:],
                                    op0=OP.subtract, op1=OP.mult)

        hT_t, ag_t = {}, {}

        def get_hT(l):
            if l not in hT_t:
                hT_t[l] = actp.tile([P, KD, TPC], BF16, name=f"hT{l}",
                                    tag="hT", bufs=2)
            return hT_t[l]

        def get_ag(l):
            if l not in ag_t:
                agin = dram.tile([NT, P, KD * P], BF16, name=f"agin{l}",
                                 tag="agin")
                agout = dram.tile([NT, 4, P, KD * P], BF16,
                                  name=f"agout{l}", tag="agout")
                ag_t[l] = (agin, agout)
            return ag_t[l]

        def emit_gather_prep(l, ts):
            """LN1 + transpose + hov for token tiles ts of layer l, sharing
            the ov stream across the tiles; fire each tile's AllGather."""
            hT = get_hT(l)
            agin, agout = get_ag(l)
            for t in ts:
                htmp = stream.tile([P, D], BF16, name="htmp", tag="lntmp")
                ln_tile(x_sb[:, t, :], htmp[:])
                for dk in range(KD):
                    transpose_to(htmp[:, dk * P:(dk + 1) * P],
                                 hT[:, dk, t * P:(t + 1) * P])
                nc.sync.dma_start(out=agin[t],
                                  in_=hT[:, :, t * P:(t + 1) * P])
                nc.gpsimd.collective_compute(
                    "AllGather", OP.bypass, replica_groups=GROUPS_BATCH,
                    ins=[agin[t].opt()], outs=[agout[t].opt()])

        def emit_hqT(l, hqT_sb):
            hT = get_hT(l)
            for j in range(KD):
                qkc = stream.tile([P, KD, P], BF16, name="qkc", tag="str2k")
                nc.scalar.dma_start(out=qkc, in_=qkt[l, j])
                mm = mm_tile()
                for k in range(KD):
                    nc.tensor.matmul(out=mm[:], lhsT=qkc[:, k, :],
                                     rhs=hT[:, k, :],
                                     start=(k == 0), stop=(k == KD - 1))
                nc.any.tensor_copy(out=hqT_sb[:, j, :], in_=mm[:])

        def emit_attn_chunk(l, m, hqT_sb, expT_sb, hnat_sb, rs_ps):
            _, agout = get_ag(l)
            qlo = m * P
            hTg = stream.tile([P, 4, KD, P], BF16, name="hTg", tag="hTg",
                              bufs=1)
            nc.sync.dma_start(
                out=hTg,
                in_=agout[m].rearrange("r p (dk n) -> p r dk n", n=P))
            for p in range(4):
                kt = 4 * m + p
                mm = mm_tile(TPC - qlo)
                for dk in range(KD):
                    nc.tensor.matmul(out=mm[:], lhsT=hTg[:, p, dk, :],
                                     rhs=hqT_sb[:, dk, qlo:TPC],
                                     start=(dk == 0), stop=(dk == KD - 1))
                dg = st3.tile([P, P], F32, name="dg", tag="str05")
                nc.vector.tensor_tensor(out=dg[:], in0=mm[:, 0:P],
                                        in1=mask_sb[:, m, p, :], op=OP.add)
                nc.scalar.activation(out=expT_sb[:, kt, qlo:qlo + P],
                                     in_=dg[:], func=AF.Exp, scale=SCALE)
                if m < NT - 1:
                    nc.scalar.activation(
                        out=expT_sb[:, kt, qlo + P:TPC],
                        in_=mm[:, P:], func=AF.Exp, scale=SCALE)
                # reconstruct natural-layout keys for the (attn @ h) matmul
                for dk in range(KD):
                    transpose_to(hTg[:, p, dk, :],
                                 hnat_sb[:, kt, dk * P:(dk + 1) * P])
            nkt = 4 * (m + 1)
            qs = slice(m * P, (m + 1) * P)
            for kt in range(nkt):
                nc.tensor.matmul(out=rs_ps[:, m:m + 1],
                                 lhsT=expT_sb[:, kt, qs], rhs=ones_t[:],
                                 start=(kt == 0), stop=(kt == nkt - 1))
            recip = small.tile([P, 1], F32, name="recip", tag="recip")
            nc.any.tensor_copy(out=recip[:], in_=rs_ps[:, m:m + 1])
            nc.vector.reciprocal(out=recip[:], in_=recip[:])
            # ypreT[d, q] = sum_t h[t, d] * expT[t, q]
            ypre = st3.tile([P, KD, P], BF16, name="ypre", tag="str2k")
            for db in range(KD):
                yp = mm_tile(P)
                for kt in range(nkt):
                    nc.tensor.matmul(
                        out=yp[:], lhsT=hnat_sb[:, kt, db * P:(db + 1) * P],
                        rhs=expT_sb[:, kt, qs],
                        start=(kt == 0), stop=(kt == nkt - 1))
                nc.any.tensor_copy(out=ypre[:, db, :], in_=yp[:])
            # y @ ov, normalize, add into x
            for c in range(2):
                ovb2 = stream.tile([P, KD, 512], BF16, name="ovb2", tag="str8k")
                nc.scalar.dma_start(
                    out=ovb2,
                    in_=ovt[l, :, :, c * 512:(c + 1) * 512]
                    .rearrange("k p n -> p k n"))
                ao = mm_tile()
                for db in range(KD):
                    nc.tensor.matmul(out=ao[:], lhsT=ypre[:, db, :],
                                     rhs=ovb2[:, db, :],
                                     start=(db == 0), stop=(db == KD - 1))
                tmp = st3.tile([P, 512], F32, name="tmp", tag="str2k")
                nc.vector.tensor_scalar_mul(out=tmp[:], in0=ao[:],
                                            scalar1=recip[:])
                nc.vector.tensor_tensor(
                    out=x_sb[:, m, c * 512:(c + 1) * 512],
                    in0=x_sb[:, m, c * 512:(c + 1) * 512],
                    in1=tmp[:], op=OP.add)

        def emit_ln2_up(l, half, mT_sb, m1T_sb, bup_sb):
            ts = (0, 1) if half == 0 else (2, 3)
            off, w = ts[0] * P, 2 * P
            for t in ts:
                mtmp = stream.tile([P, D], BF16, name="mtmp", tag="lntmp")
                ln_tile(x_sb[:, t, :], mtmp[:])
                for dk in range(KD):
                    transpose_to(mtmp[:, dk * P:(dk + 1) * P],
                                 mT_sb[:, dk, t * P:(t + 1) * P])
            for hg in range(KH // 4):
                wt = stream.tile([P, 4, KD, P], BF16, name="wupt", tag="str8k")
                nc.scalar.dma_start(
                    out=wt,
                    in_=wup[l, 4 * hg:4 * hg + 4]
                    .rearrange("h p k n -> p h k n"))
                for hi in range(4):
                    ht = 4 * hg + hi
                    mm = mm_tile(w)
                    for k in range(KD):
                        nc.tensor.matmul(out=mm[:], lhsT=wt[:, hi, k, :],
                                         rhs=mT_sb[:, k, off:off + w],
                                         start=(k == 0), stop=(k == KD - 1))
                    nc.scalar.activation(out=m1T_sb[:, ht, off:off + w],
                                         in_=mm[:], func=AF.Lrelu,
                                         bias=bup_sb[:, ht:ht + 1],
                                         alpha=LRELU)

        def emit_hid(l, half, m1T_sb, m2T_sb, bhid_sb):
            off, w = half * 2 * P, 2 * P
            for ht in range(KH):
                wt = stream.tile([P, KH, P], BF16, name="whidt", tag="str8k")
                nc.scalar.dma_start(out=wt, in_=whid[l, ht])
                mm = mm_tile(w)
                for k in range(KH):
                    nc.tensor.matmul(out=mm[:], lhsT=wt[:, k, :],
                                     rhs=m1T_sb[:, k, off:off + w],
                                     start=(k == 0), stop=(k == KH - 1))
                nc.scalar.activation(out=m2T_sb[:, ht, off:off + w], in_=mm[:],
                                     func=AF.Lrelu, bias=bhid_sb[:, ht:ht + 1],
                                     alpha=LRELU)

        def emit_down(l, half, m2T_sb):
            ts = (0, 1) if half == 0 else (2, 3)
            for c in range(2):
                psl = {t: mm_tile() for t in ts}
                for kg in range(KH // 4):
                    wt = stream.tile([P, 4, 512], BF16, name="wdnt", tag="str8k")
                    nc.scalar.dma_start(
                        out=wt,
                        in_=wdn[l, 4 * kg:4 * kg + 4, :, c * 512:(c + 1) * 512]
                        .rearrange("k p n -> p k n"))
                    for ki in range(4):
                        k = 4 * kg + ki
                        for t in ts:
                            nc.tensor.matmul(
                                out=psl[t][:],
                                lhsT=m2T_sb[:, k, t * P:(t + 1) * P],
                                rhs=wt[:, ki, :], start=(k == 0),
                                stop=(k == KH - 1))
                for t in ts:
                    m3 = st3.tile([P, 512], F32, name="m3ev", tag="str2k")
                    nc.scalar.activation(out=m3[:], in_=psl[t][:], func=AF.Lrelu,
                                         alpha=LRELU)
                    nc.vector.tensor_tensor(
                        out=x_sb[:, t, c * 512:(c + 1) * 512],
                        in0=x_sb[:, t, c * 512:(c + 1) * 512],
                        in1=m3[:], op=OP.add)

        xin = None
        xoutg = {}

        def emit_final_gather(ts):
            nonlocal xin
            if xin is None:
                xin = dram.tile([NT, P, KD * P], BF16, name="xin", tag="xin",
                                bufs=1)
            for t in ts:
                xtmp = stream.tile([P, D], BF16, name="xtmp", tag="lntmp")
                nc.any.tensor_copy(out=xtmp[:], in_=x_sb[:, t, :])
                xT1 = stream.tile([P, KD, P], BF16, name="xT1", tag="str2k")
                for dk in range(KD):
                    transpose_to(xtmp[:, dk * P:(dk + 1) * P], xT1[:, dk, :])
                xo = dram.tile([NCORES, P, KD * P], BF16, name=f"xoutg{t}",
                               tag=f"xoutg{t}", addr_space="Shared", bufs=1)
                xoutg[t] = xo
                nc.sync.dma_start(out=xin[t], in_=xT1[:])
                nc.gpsimd.collective_compute(
                    "AllGather", OP.bypass, replica_groups=GROUP_ALL,
                    ins=[xin[t].opt()], outs=[xo.opt()])

        def emit_unembed():
            emb_lo = bigp.tile([P, KD // 2, VSH], BF16, name="emb_lo",
                               tag="m1T")
            emb_hi = bigp.tile([P, KD // 2, VSH], BF16, name="emb_hi",
                               tag="m2T")
            nc.sync.dma_start(
                out=emb_lo,
                in_=embT[0:D // 2].rearrange("(dk p) v -> p dk v", p=P))
            nc.sync.dma_start(
                out=emb_hi,
                in_=embT[D // 2:D].rearrange("(dk p) v -> p dk v", p=P))

            def embT_tile(dk, vs):
                src = emb_lo if dk < 4 else emb_hi
                return src[:, dk % 4, vs]

            for t in range(NT):
                for r in range(NCORES):
                    xtg = stream.tile([P, KD, P], BF16, name="xtg", tag="str2k")
                    nc.sync.dma_start(
                        out=xtg,
                        in_=xoutg[t][r].rearrange("p (dk n) -> p dk n", n=P))
                    grow = ((r // 4) * S + (4 * t + (r % 4)) * P)
                    lg = stream.tile([P, VSH], BF16, name="lg", tag="str8k")
                    for v in range(NVC):
                        vs = slice(v * VC, (v + 1) * VC)
                        mm = mm_tile(VC)
                        for dk in range(KD):
                            nc.tensor.matmul(out=mm[:], lhsT=xtg[:, dk, :],
                                             rhs=embT_tile(dk, vs),
                                             start=(dk == 0),
                                             stop=(dk == KD - 1))
                        nc.any.tensor_copy(out=lg[:, vs], in_=mm[:])
                    nc.sync.dma_start(out=logits[grow:grow + P, :], in_=lg[:])

        # ---------------- pipeline driver ----------------
        emit_gather_prep(0, (0, 1))
        emit_gather_prep(0, (2, 3))
        for l in range(L):
            hqT_sb = actp.tile([P, KD, TPC], BF16, name=f"hqT{l}", tag="hqT")
            expT_sb = bigp.tile([P, 16, TPC], BF16, name=f"expT{l}", tag="expT")
            hnat_sb = bigp.tile([P, 16, D], BF16, name=f"hnat{l}", tag="hovg")
            rs_ps = ps.tile([P, NT], F32, name=f"rs{l}", tag="rs", bufs=1)
            mT_sb = actp.tile([P, KD, TPC], BF16, name=f"mT{l}", tag="hT",
                              bufs=2)
            m1T_sb = bigp.tile([P, KH, TPC], BF16, name=f"m1T{l}", tag="m1T")
            m2T_sb = bigp.tile([P, KH, TPC], BF16, name=f"m2T{l}", tag="m2T")
            bup_sb = small.tile([P, KH], F32, name="bup_sb", tag="bup")
            nc.sync.dma_start(out=bup_sb,
                              in_=bup_all[l].rearrange("(ht p) -> p ht", p=P))
            bhid_sb = small.tile([P, KH], F32, name="bhid_sb", tag="bhid")
            nc.sync.dma_start(out=bhid_sb,
                              in_=bhid_all[l].rearrange("(ht p) -> p ht", p=P))

            emit_hqT(l, hqT_sb)
            emit_attn_chunk(l, 0, hqT_sb, expT_sb, hnat_sb, rs_ps)
            emit_attn_chunk(l, 1, hqT_sb, expT_sb, hnat_sb, rs_ps)
            emit_ln2_up(l, 0, mT_sb, m1T_sb, bup_sb)
            emit_hid(l, 0, m1T_sb, m2T_sb, bhid_sb)
            emit_down(l, 0, m2T_sb)
            if l < L - 1:
                emit_gather_prep(l + 1, (0, 1))
            else:
                emit_final_gather((0, 1))
            emit_attn_chunk(l, 2, hqT_sb, expT_sb, hnat_sb, rs_ps)
            emit_attn_chunk(l, 3, hqT_sb, expT_sb, hnat_sb, rs_ps)
            emit_ln2_up(l, 1, mT_sb, m1T_sb, bup_sb)
            emit_hid(l, 1, m1T_sb, m2T_sb, bhid_sb)
            emit_down(l, 1, m2T_sb)
            if l < L - 1:
                emit_gather_prep(l + 1, (2, 3))
            else:
                emit_final_gather((2, 3))
        emit_unembed()

    nc.compile()
    return nc


_CACHE = {}


def _get_program():
    if "nc" not in _CACHE:
        _CACHE["nc"] = build_program()
    return _CACHE["nc"]


def _bf16(x):
    import ml_dtypes
    return np.ascontiguousarray(x.astype(ml_dtypes.bfloat16))


def _core_token_index(core):
    b, lc = core // 4, core % 4
    idx = []
    for j in range(NT):
        g = 4 * j + lc
        idx.append(b * S + g * P + np.arange(P))
    return np.concatenate(idx)


def _make_maskT(core):
    lc = core % 4
    out = np.zeros((NT, 4, P, P), np.float32)
    k = np.arange(P)[:, None]
    q = np.arange(P)[None, :]
    for j in range(NT):
        for p in range(4):
            if p < lc:
                continue
            elif p == lc:
                out[j, p] = np.where(k <= q, 0.0, MASKV)
            else:
                out[j, p] = MASKV
    return out


def kernel(**inputs):
    tokens = np.asarray(inputs["tokens"]).astype(np.int64).reshape(T)
    emb = np.ascontiguousarray(np.asarray(inputs["emb"], dtype=np.float32))
    qk = np.asarray(inputs["qk"], dtype=np.float32)
    ov = np.asarray(inputs["ov"], dtype=np.float32)
    w_up = np.asarray(inputs["w_up"], dtype=np.float32)
    w_hid = np.asarray(inputs["w_hid"], dtype=np.float32)
    w_down = np.asarray(inputs["w_down"], dtype=np.float32)
    b_up = np.ascontiguousarray(np.asarray(inputs["b_up"], dtype=np.float32))
    b_hid = np.ascontiguousarray(np.asarray(inputs["b_hid"], dtype=np.float32))

    # swizzles: [l, out-block, partition(k%128), k-block, out-within-block]
    qkt = _bf16(qk.reshape(L, KD, P, KD, P).transpose(0, 3, 2, 1, 4))
    ovt = _bf16(ov.reshape(L, KD, P, D))
    wupt = _bf16(w_up.reshape(L, KD, P, KH, P).transpose(0, 3, 2, 1, 4))
    whidt = _bf16(w_hid.reshape(L, KH, P, KH, P).transpose(0, 3, 2, 1, 4))
    wdnt = _bf16(w_down.reshape(L, KH, P, D))

    nc = _get_program()
    in_maps = []
    for core in range(NCORES):
        tidx = _core_token_index(core)
        in_maps.append({
            "x0": np.ascontiguousarray(emb[tokens[tidx]]),
            "maskT": _make_maskT(core),
            "qkt": qkt, "ovt": ovt,
            "wup": wupt, "whid": whidt, "wdn": wdnt,
            "bup": b_up, "bhid": b_hid,
            "embT": _bf16(emb[core * VSH:(core + 1) * VSH].T),
        })
    res = run_bass_kernel_spmd(nc, in_maps, core_ids=list(range(NCORES)))
    _CACHE["last"] = res
    shards = [res.results[c]["logits"] for c in range(NCORES)]
    fullb = np.concatenate(shards, axis=1)
    full = np.empty((T, V), np.float32)
    fv = full.view(np.uint32)
    fv[:] = fullb.view(np.uint16).astype(np.uint32) << 16
    return full.reshape(B, S, V)


# revision 3
# speedup vs baseline: 1.0084x; 1.0084x over previous
"""Trainium2 Bass kernel v4 (self-contained).

v2 math (causal slotting, transposed scores, bf16 operands) + v3 pipeline,
restructured to keep whid streamed once per layer:

  layer l body:
    hqT(l) | attn 0 | attn 1 | LN2A+upA | attn 2 | attn 3 | upB
    | hid (full, single whid stream) | downA | LN1+hov+fire(l+1, tiles 0,1)
    | downB | LN1+hov+fire(l+1, tiles 2,3)
  last layer fires the 8-core Shared xT gathers after each down half;
  unembed drains them.

All weight DRAM layouts are pre-swizzled so every SBUF tile load is >=1KB
contiguous per partition (DMA descriptor efficiency).
"""

import numpy as np
from contextlib import ExitStack

import concourse.bass as bass
import concourse.bacc as bacc
import concourse.tile as tile
from concourse import mybir
from concourse.bass_utils import run_bass_kernel_spmd
from concourse.masks import make_identity

F32 = mybir.dt.float32
BF16 = mybir.dt.bfloat16
AF = mybir.ActivationFunctionType
OP = mybir.AluOpType

P = 128
D = 1024
H = 4096
L = 4
V = 32000
B = 2
S = 2048
T = B * S
NCORES = 8
TPC = T // NCORES    # 512
VSH = V // NCORES    # 4000
KD = D // P          # 8
KH = H // P          # 32
NT = TPC // P        # 4
NVC = 8
VC = VSH // NVC      # 500
LRELU = 0.01
SCALE = 1.0 / float(np.sqrt(D))
MASKV = -1e9
GROUPS_BATCH = [[0, 1, 2, 3], [4, 5, 6, 7]]
GROUP_ALL = [[0, 1, 2, 3, 4, 5, 6, 7]]


def build_program():
    nc = bacc.Bacc(None, num_devices=NCORES)

    x0 = nc.dram_tensor("x0", [TPC, D], F32, kind="ExternalInput")
    h0s = nc.dram_tensor("h0s", [NT, P, KD * P], BF16, kind="ExternalInput")
    maskT = nc.dram_tensor("maskT", [NT, 4, P, P], F32, kind="ExternalInput")
    # pre-swizzled weights: inner dims contiguous per partition
    qkt = nc.dram_tensor("qkt", [L, KD, P, KD, P], BF16, kind="ExternalInput")
    ovt = nc.dram_tensor("ovt", [L, KD, P, D], BF16, kind="ExternalInput")
    wup = nc.dram_tensor("wup", [L, KH, P, KD, P], BF16, kind="ExternalInput")
    whid = nc.dram_tensor("whid", [L, KH, P, KH, P], BF16, kind="ExternalInput")
    wdn = nc.dram_tensor("wdn", [L, KH, P, D], BF16, kind="ExternalInput")
    bup_all = nc.dram_tensor("bup", [L, H], F32, kind="ExternalInput")
    bhid_all = nc.dram_tensor("bhid", [L, H], F32, kind="ExternalInput")
    embT = nc.dram_tensor("embT", [D, VSH], BF16, kind="ExternalInput")
    logits = nc.dram_tensor("logits", [T, VSH], BF16, kind="ExternalOutput")

    with tile.TileContext(nc) as tc, ExitStack() as ctx:
        const = ctx.enter_context(tc.tile_pool(name="const", bufs=1))
        state = ctx.enter_context(tc.tile_pool(name="state", bufs=1))
        actp = ctx.enter_context(tc.tile_pool(name="actp", bufs=1))
        bigp = ctx.enter_context(tc.tile_pool(name="bigp", bufs=1))
        small = ctx.enter_context(tc.tile_pool(name="small", bufs=4))
        stream = ctx.enter_context(tc.tile_pool(name="stream", bufs=2))
        st3 = ctx.enter_context(tc.tile_pool(name="st3", bufs=3))
        ps = ctx.enter_context(tc.tile_pool(name="ps", bufs=1, space="PSUM"))
        dram = ctx.enter_context(tc.tile_pool(name="dram", bufs=2, space="DRAM"))

        identB = const.tile([P, P], BF16)
        make_identity(nc, identB[:])
        eps_t = const.tile([P, 1], F32)
        nc.vector.memset(eps_t, 1e-5)
        ones_t = const.tile([P, 1], BF16)
        nc.vector.memset(ones_t, 1.0)
        mask_sb = const.tile([P, NT, 4, P], F32)
        nc.sync.dma_start(out=mask_sb,
                          in_=maskT.rearrange("j t k q -> k j t q"))

        x_sb = state.tile([P, NT, D], F32)
        nc.sync.dma_start(out=x_sb,
                          in_=x0.rearrange("(tt p) d -> p tt d", p=P))

        def transpose_to(src_block, dst_ap):
            tp = ps.tile([P, P], BF16, name="trps", tag="tr", bufs=2)
            nc.tensor.transpose(out=tp[:], in_=src_block, identity=identB[:])
            nc.any.tensor_copy(out=dst_ap, in_=tp[:])

        def mm_tile(cols=512):
            return ps.tile([P, cols], F32, name="mm", tag="mm", bufs=5,
                           padded_shape=[P, 512])

        def ln_tile(x_ap, out_ap):
            stats = small.tile([P, 2, nc.vector.BN_STATS_DIM], F32,
                               name="lnst", tag="lnst")
            xv = x_ap.rearrange("p (s f) -> p s f", s=2)
            for s in range(2):
                nc.vector.bn_stats(out=stats[:, s, :], in_=xv[:, s, :])
            mv = small.tile([P, nc.vector.BN_AGGR_DIM], F32, name="lnmv",
                            tag="lnmv")
            nc.vector.bn_aggr(out=mv[:], in_=stats[:])
            rstd = small.tile([P, 1], F32, name="lnrs", tag="lnrs")
            nc.scalar.activation(out=rstd[:], in_=mv[:, 1:2], func=AF.Sqrt,
                                 bias=eps_t[:])
            nc.vector.reciprocal(out=rstd[:], in_=rstd[:])
            nc.vector.tensor_scalar(out=out_ap, in0=x_ap,
                                    scalar1=mv[:, 0:1], scalar2=rstd[:],
                                    op0=OP.subtract, op1=OP.mult)

        hT_t, ag_t = {}, {}

        def get_hT(l):
            if l not in hT_t:
                hT_t[l] = actp.tile([P, KD, TPC], BF16, name=f"hT{l}",
                                    tag="hT", bufs=2)
            return hT_t[l]

        def get_ag(l):
            if l not in ag_t:
                agin = dram.tile([NT, P, KD * P], BF16, name=f"agin{l}",
                                 tag="agin")
                agout = dram.tile([NT, 4, P, KD * P], BF16,
                                  name=f"agout{l}", tag="agout")
                ag_t[l] = (agin, agout)
            return ag_t[l]

        def emit_gather_prep(l, ts):
            """LN1 + transpose + hov for token tiles ts of layer l, sharing
            the ov stream across the tiles; fire each tile's AllGather."""
            hT = get_hT(l)
            agin, agout = get_ag(l)
            for t in ts:
                htmp = stream.tile([P, D], BF16, name="htmp", tag="lntmp")
                ln_tile(x_sb[:, t, :], htmp[:])
                for dk in range(KD):
                    transpose_to(htmp[:, dk * P:(dk + 1) * P],
                                 hT[:, dk, t * P:(t + 1) * P])
                nc.sync.dma_start(out=agin[t],
                                  in_=hT[:, :, t * P:(t + 1) * P])
                nc.gpsimd.collective_compute(
                    "AllGather", OP.bypass, replica_groups=GROUPS_BATCH,
                    ins=[agin[t].opt()], outs=[agout[t].opt()])

        def emit_hqT(l, hqT_sb):
            hT = get_hT(l)
            for j in range(KD):
                qkc = stream.tile([P, KD, P], BF16, name="qkc", tag="str2k")
                nc.scalar.dma_start(out=qkc, in_=qkt[l, j])
                mm = mm_tile()
                for k in range(KD):
                    nc.tensor.matmul(out=mm[:], lhsT=qkc[:, k, :],
                                     rhs=hT[:, k, :],
                                     start=(k == 0), stop=(k == KD - 1))
                nc.any.tensor_copy(out=hqT_sb[:, j, :], in_=mm[:])

        def emit_attn_chunk(l, m, hqT_sb, expT_sb, hnat_sb, rs_ps):
            _, agout = get_ag(l)
            qlo = m * P
            hTg = stream.tile([P, 4, KD, P], BF16, name="hTg", tag="hTg",
                              bufs=1)
            nc.sync.dma_start(
                out=hTg,
                in_=agout[m].rearrange("r p (dk n) -> p r dk n", n=P))
            for p in range(4):
                kt = 4 * m + p
                mm = mm_tile(TPC - qlo)
                for dk in range(KD):
                    nc.tensor.matmul(out=mm[:], lhsT=hTg[:, p, dk, :],
                                     rhs=hqT_sb[:, dk, qlo:TPC],
                                     start=(dk == 0), stop=(dk == KD - 1))
                dg = st3.tile([P, P], F32, name="dg", tag="str05")
                nc.vector.tensor_tensor(out=dg[:], in0=mm[:, 0:P],
                                        in1=mask_sb[:, m, p, :], op=OP.add)
                nc.scalar.activation(out=expT_sb[:, kt, qlo:qlo + P],
                                     in_=dg[:], func=AF.Exp, scale=SCALE)
                if m < NT - 1:
                    nc.scalar.activation(
                        out=expT_sb[:, kt, qlo + P:TPC],
                        in_=mm[:, P:], func=AF.Exp, scale=SCALE)
                # reconstruct natural-layout keys for the (attn @ h) matmul
                for dk in range(KD):
                    transpose_to(hTg[:, p, dk, :],
                                 hnat_sb[:, kt, dk * P:(dk + 1) * P])
            nkt = 4 * (m + 1)
            qs = slice(m * P, (m + 1) * P)
            for kt in range(nkt):
                nc.tensor.matmul(out=rs_ps[:, m:m + 1],
                                 lhsT=expT_sb[:, kt, qs], rhs=ones_t[:],
                                 start=(kt == 0), stop=(kt == nkt - 1))
            recip = small.tile([P, 1], F32, name="recip", tag="recip")
            nc.any.tensor_copy(out=recip[:], in_=rs_ps[:, m:m + 1])
            nc.vector.reciprocal(out=recip[:], in_=recip[:])
            # ypreT[d, q] = sum_t h[t, d] * expT[t, q]
            ypre = st3.tile([P, KD, P], BF16, name="ypre", tag="str2k")
            for db in range(KD):
                yp = mm_tile(P)
                for kt in range(nkt):
                    nc.tensor.matmul(
                        out=yp[:], lhsT=hnat_sb[:, kt, db * P:(db + 1) * P],
                        rhs=expT_sb[:, kt, qs],
                        start=(kt == 0), stop=(kt == nkt - 1))
                nc.any.tensor_copy(out=ypre[:, db, :], in_=yp[:])
            # y @ ov, normalize, add into x
            for c in range(2):
                ovb2 = stream.tile([P, KD, 512], BF16, name="ovb2", tag="str8k")
                nc.scalar.dma_start(
                    out=ovb2,
                    in_=ovt[l, :, :, c * 512:(c + 1) * 512]
                    .rearrange("k p n -> p k n"))
                ao = mm_tile()
                for db in range(KD):
                    nc.tensor.matmul(out=ao[:], lhsT=ypre[:, db, :],
                                     rhs=ovb2[:, db, :],
                                     start=(db == 0), stop=(db == KD - 1))
                tmp = st3.tile([P, 512], F32, name="tmp", tag="str2k")
                nc.vector.tensor_scalar_mul(out=tmp[:], in0=ao[:],
                                            scalar1=recip[:])
                nc.vector.tensor_tensor(
                    out=x_sb[:, m, c * 512:(c + 1) * 512],
                    in0=x_sb[:, m, c * 512:(c + 1) * 512],
                    in1=tmp[:], op=OP.add)

        def emit_ln2_up(l, half, mT_sb, m1T_sb, bup_sb):
            ts = (0, 1) if half == 0 else (2, 3)
            off, w = ts[0] * P, 2 * P
            for t in ts:
                mtmp = stream.tile([P, D], BF16, name="mtmp", tag="lntmp")
                ln_tile(x_sb[:, t, :], mtmp[:])
                for dk in range(KD):
                    transpose_to(mtmp[:, dk * P:(dk + 1) * P],
                                 mT_sb[:, dk, t * P:(t + 1) * P])
            for hg in range(KH // 4):
                wt = stream.tile([P, 4, KD, P], BF16, name="wupt", tag="str8k")
                nc.scalar.dma_start(
                    out=wt,
                    in_=wup[l, 4 * hg:4 * hg + 4]
                    .rearrange("h p k n -> p h k n"))
                for hi in range(4):
                    ht = 4 * hg + hi
                    mm = mm_tile(w)
                    for k in range(KD):
                        nc.tensor.matmul(out=mm[:], lhsT=wt[:, hi, k, :],
                                         rhs=mT_sb[:, k, off:off + w],
                                         start=(k == 0), stop=(k == KD - 1))
                    nc.scalar.activation(out=m1T_sb[:, ht, off:off + w],
                                         in_=mm[:], func=AF.Lrelu,
                                         bias=bup_sb[:, ht:ht + 1],
                                         alpha=LRELU)

        def emit_hid(l, half, m1T_sb, m2T_sb, bhid_sb):
            off, w = half * 2 * P, 2 * P
            for ht in range(KH):
                wt = stream.tile([P, KH, P], BF16, name="whidt", tag="str8k")
                nc.scalar.dma_start(out=wt, in_=whid[l, ht])
                mm = mm_tile(w)
                for k in range(KH):
                    nc.tensor.matmul(out=mm[:], lhsT=wt[:, k, :],
                                     rhs=m1T_sb[:, k, off:off + w],
                                     start=(k == 0), stop=(k == KH - 1))
                nc.scalar.activation(out=m2T_sb[:, ht, off:off + w], in_=mm[:],
                                     func=AF.Lrelu, bias=bhid_sb[:, ht:ht + 1],
                                     alpha=LRELU)

        def emit_down(l, half, m2T_sb):
            ts = (0, 1) if half == 0 else (2, 3)
            for c in range(2):
                psl = {t: mm_tile() for t in ts}
                for kg in range(KH // 4):
                    wt = stream.tile([P, 4, 512], BF16, name="wdnt", tag="str8k")
                    nc.scalar.dma_start(
                        out=wt,
                        in_=wdn[l, 4 * kg:4 * kg + 4, :, c * 512:(c + 1) * 512]
                        .rearrange("k p n -> p k n"))
                    for ki in range(4):
                        k = 4 * kg + ki
                        for t in ts:
                            nc.tensor.matmul(
                                out=psl[t][:],
                                lhsT=m2T_sb[:, k, t * P:(t + 1) * P],
                                rhs=wt[:, ki, :], start=(k == 0),
                                stop=(k == KH - 1))
                for t in ts:
                    m3 = st3.tile([P, 512], F32, name="m3ev", tag="str2k")
                    nc.scalar.activation(out=m3[:], in_=psl[t][:], func=AF.Lrelu,
                                         alpha=LRELU)
                    nc.vector.tensor_tensor(
                        out=x_sb[:, t, c * 512:(c + 1) * 512],
                        in0=x_sb[:, t, c * 512:(c + 1) * 512],
                        in1=m3[:], op=OP.add)

        xin = None
        xoutg = {}

        def emit_final_gather(ts):
            nonlocal xin
            if xin is None:
                xin = dram.tile([NT, P, KD * P], BF16, name="xin", tag="xin",
                                bufs=1)
            for t in ts:
                xtmp = stream.tile([P, D], BF16, name="xtmp", tag="lntmp")
                nc.any.tensor_copy(out=xtmp[:], in_=x_sb[:, t, :])
                xT1 = stream.tile([P, KD, P], BF16, name="xT1", tag="str2k")
                for dk in range(KD):
                    transpose_to(xtmp[:, dk * P:(dk + 1) * P], xT1[:, dk, :])
                xo = dram.tile([NCORES, P, KD * P], BF16, name=f"xoutg{t}",
                               tag=f"xoutg{t}", addr_space="Shared", bufs=1)
                xoutg[t] = xo
                nc.sync.dma_start(out=xin[t], in_=xT1[:])
                nc.gpsimd.collective_compute(
                    "AllGather", OP.bypass, replica_groups=GROUP_ALL,
                    ins=[xin[t].opt()], outs=[xo.opt()])

        def emit_unembed():
            emb_lo = bigp.tile([P, KD // 2, VSH], BF16, name="emb_lo",
                               tag="m1T")
            emb_hi = bigp.tile([P, KD // 2, VSH], BF16, name="emb_hi",
                               tag="m2T")
            nc.sync.dma_start(
                out=emb_lo,
                in_=embT[0:D // 2].rearrange("(dk p) v -> p dk v", p=P))
            nc.sync.dma_start(
                out=emb_hi,
                in_=embT[D // 2:D].rearrange("(dk p) v -> p dk v", p=P))

            def embT_tile(dk, vs):
                src = emb_lo if dk < 4 else emb_hi
                return src[:, dk % 4, vs]

            for t in range(NT):
                for r in range(NCORES):
                    xtg = stream.tile([P, KD, P], BF16, name="xtg", tag="str2k")
                    nc.sync.dma_start(
                        out=xtg,
                        in_=xoutg[t][r].rearrange("p (dk n) -> p dk n", n=P))
                    grow = ((r // 4) * S + (4 * t + (r % 4)) * P)
                    lg = stream.tile([P, VSH], BF16, name="lg", tag="str8k")
                    for v in range(NVC):
                        vs = slice(v * VC, (v + 1) * VC)
                        mm = mm_tile(VC)
                        for dk in range(KD):
                            nc.tensor.matmul(out=mm[:], lhsT=xtg[:, dk, :],
                                             rhs=embT_tile(dk, vs),
                                             start=(dk == 0),
                                             stop=(dk == KD - 1))
                        nc.any.tensor_copy(out=lg[:, vs], in_=mm[:])
                    nc.sync.dma_start(out=logits[grow:grow + P, :], in_=lg[:])

        # ---------------- pipeline driver ----------------
        agin0, agout0 = get_ag(0)
        for t in range(NT):
            nc.sync.dma_start(out=agin0[t], in_=h0s[t])
            nc.gpsimd.collective_compute(
                "AllGather", OP.bypass, replica_groups=GROUPS_BATCH,
                ins=[agin0[t].opt()], outs=[agout0[t].opt()])
        hT0 = get_hT(0)
        nc.sync.dma_start(
            out=hT0.rearrange("p dk (t n) -> p dk t n", n=P),
            in_=h0s.rearrange("t p (dk n) -> p dk t n", n=P))
        for l in range(L):
            hqT_sb = actp.tile([P, KD, TPC], BF16, name=f"hqT{l}", tag="hqT")
            expT_sb = bigp.tile([P, 16, TPC], BF16, name=f"expT{l}", tag="expT")
            hnat_sb = bigp.tile([P, 16, D], BF16, name=f"hnat{l}", tag="hovg")
            rs_ps = ps.tile([P, NT], F32, name=f"rs{l}", tag="rs", bufs=1)
            mT_sb = actp.tile([P, KD, TPC], BF16, name=f"mT{l}", tag="hT",
                              bufs=2)
            m1T_sb = bigp.tile([P, KH, TPC], BF16, name=f"m1T{l}", tag="m1T")
            m2T_sb = bigp.tile([P, KH, TPC], BF16, name=f"m2T{l}", tag="m2T")
            bup_sb = small.tile([P, KH], F32, name="bup_sb", tag="bup")
            nc.sync.dma_start(out=bup_sb,
                              in_=bup_all[l].rearrange("(ht p) -> p ht", p=P))
            bhid_sb = small.tile([P, KH], F32, name="bhid_sb", tag="bhid")
            nc.sync.dma_start(out=bhid_sb,
                              in_=bhid_all[l].rearrange("(ht p) -> p ht", p=P))

            emit_hqT(l, hqT_sb)
            emit_attn_chunk(l, 0, hqT_sb, expT_sb, hnat_sb, rs_ps)
            emit_attn_chunk(l, 1, hqT_sb, expT_sb, hnat_sb, rs_ps)
            emit_ln2_up(l, 0, mT_sb, m1T_sb, bup_sb)
            emit_hid(l, 0, m1T_sb, m2T_sb, bhid_sb)
            emit_down(l, 0, m2T_sb)
            if l < L - 1:
                emit_gather_prep(l + 1, (0, 1))
            else:
                emit_final_gather((0, 1))
            emit_attn_chunk(l, 2, hqT_sb, expT_sb, hnat_sb, rs_ps)
            emit_attn_chunk(l, 3, hqT_sb, expT_sb, hnat_sb, rs_ps)
            emit_ln2_up(l, 1, mT_sb, m1T_sb, bup_sb)
            emit_hid(l, 1, m1T_sb, m2T_sb, bhid_sb)
            emit_down(l, 1, m2T_sb)
            if l < L - 1:
                emit_gather_prep(l + 1, (2, 3))
            else:
                emit_final_gather((2, 3))
        emit_unembed()

    nc.compile()
    return nc


_CACHE = {}


def _get_program():
    if "nc" not in _CACHE:
        _CACHE["nc"] = build_program()
    return _CACHE["nc"]


def _bf16(x):
    import ml_dtypes
    return np.ascontiguousarray(x.astype(ml_dtypes.bfloat16))


def _core_token_index(core):
    b, lc = core // 4, core % 4
    idx = []
    for j in range(NT):
        g = 4 * j + lc
        idx.append(b * S + g * P + np.arange(P))
    return np.concatenate(idx)


def _make_maskT(core):
    lc = core % 4
    out = np.zeros((NT, 4, P, P), np.float32)
    k = np.arange(P)[:, None]
    q = np.arange(P)[None, :]
    for j in range(NT):
        for p in range(4):
            if p < lc:
                continue
            elif p == lc:
                out[j, p] = np.where(k <= q, 0.0, MASKV)
            else:
                out[j, p] = MASKV
    return out


def kernel(**inputs):
    tokens = np.asarray(inputs["tokens"]).astype(np.int64).reshape(T)
    emb = np.ascontiguousarray(np.asarray(inputs["emb"], dtype=np.float32))
    qk = np.asarray(inputs["qk"], dtype=np.float32)
    ov = np.asarray(inputs["ov"], dtype=np.float32)
    w_up = np.asarray(inputs["w_up"], dtype=np.float32)
    w_hid = np.asarray(inputs["w_hid"], dtype=np.float32)
    w_down = np.asarray(inputs["w_down"], dtype=np.float32)
    b_up = np.ascontiguousarray(np.asarray(inputs["b_up"], dtype=np.float32))
    b_hid = np.ascontiguousarray(np.asarray(inputs["b_hid"], dtype=np.float32))

    # swizzles: [l, out-block, partition(k%128), k-block, out-within-block]
    qkt = _bf16(qk.reshape(L, KD, P, KD, P).transpose(0, 3, 2, 1, 4))
    ovt = _bf16(ov.reshape(L, KD, P, D))
    wupt = _bf16(w_up.reshape(L, KD, P, KH, P).transpose(0, 3, 2, 1, 4))
    whidt = _bf16(w_hid.reshape(L, KH, P, KH, P).transpose(0, 3, 2, 1, 4))
    wdnt = _bf16(w_down.reshape(L, KH, P, D))

    nc = _get_program()
    in_maps = []
    for core in range(NCORES):
        tidx = _core_token_index(core)
        x0c = np.ascontiguousarray(emb[tokens[tidx]])
        mu = x0c.mean(-1, keepdims=True)
        var = ((x0c - mu) ** 2).mean(-1, keepdims=True)
        h0 = (x0c - mu) / np.sqrt(var + 1e-5)
        h0s = _bf16(h0.reshape(NT, P, KD, P).transpose(0, 3, 2, 1)
                    .reshape(NT, P, KD * P))
        in_maps.append({
            "x0": x0c,
            "h0s": h0s,
            "maskT": _make_maskT(core),
            "qkt": qkt, "ovt": ovt,
            "wup": wupt, "whid": whidt, "wdn": wdnt,
            "bup": b_up, "bhid": b_hid,
            "embT": _bf16(emb[core * VSH:(core + 1) * VSH].T),
        })
    res = run_bass_kernel_spmd(nc, in_maps, core_ids=list(range(NCORES)))
    _CACHE["last"] = res
    shards = [res.results[c]["logits"] for c in range(NCORES)]
    fullb = np.concatenate(shards, axis=1)
    full = np.empty((T, V), np.float32)
    fv = full.view(np.uint32)
    fv[:] = fullb.view(np.uint16).astype(np.uint32) << 16
    return full.reshape(B, S, V)


# revision 4
# speedup vs baseline: 1.0389x; 1.0302x over previous
"""Trainium2 Bass kernel v4 (self-contained).

v2 math (causal slotting, transposed scores, bf16 operands) + v3 pipeline,
restructured to keep whid streamed once per layer:

  layer l body:
    hqT(l) | attn 0 | attn 1 | LN2A+upA | attn 2 | attn 3 | upB
    | hid (full, single whid stream) | downA | LN1+hov+fire(l+1, tiles 0,1)
    | downB | LN1+hov+fire(l+1, tiles 2,3)
  last layer fires the 8-core Shared xT gathers after each down half;
  unembed drains them.

All weight DRAM layouts are pre-swizzled so every SBUF tile load is >=1KB
contiguous per partition (DMA descriptor efficiency).
"""

import numpy as np
from contextlib import ExitStack

import concourse.bass as bass
import concourse.bacc as bacc
import concourse.tile as tile
from concourse import mybir
from concourse.bass_utils import run_bass_kernel_spmd
from concourse.masks import make_identity

F32 = mybir.dt.float32
BF16 = mybir.dt.bfloat16
AF = mybir.ActivationFunctionType
OP = mybir.AluOpType

P = 128
D = 1024
H = 4096
L = 4
V = 32000
B = 2
S = 2048
T = B * S
NCORES = 8
TPC = T // NCORES    # 512
VSH = V // NCORES    # 4000
KD = D // P          # 8
KH = H // P          # 32
NT = TPC // P        # 4
NVC = 8
VC = VSH // NVC      # 500
LRELU = 0.01
SCALE = 1.0 / float(np.sqrt(D))
MASKV = -1e9
GROUPS_BATCH = [[0, 1, 2, 3], [4, 5, 6, 7]]
GROUP_ALL = [[0, 1, 2, 3, 4, 5, 6, 7]]


def build_program():
    nc = bacc.Bacc(None, num_devices=NCORES)

    x0 = nc.dram_tensor("x0", [TPC, D], F32, kind="ExternalInput")
    h0s = nc.dram_tensor("h0s", [NT, P, KD * P], BF16, kind="ExternalInput")
    maskT = nc.dram_tensor("maskT", [NT, 4, P, P], F32, kind="ExternalInput")
    # pre-swizzled weights: inner dims contiguous per partition
    qkt = nc.dram_tensor("qkt", [L, KD, P, KD, P], BF16, kind="ExternalInput")
    ovt = nc.dram_tensor("ovt", [L, KD, P, D], BF16, kind="ExternalInput")
    wup = nc.dram_tensor("wup", [L, KH, P, KD, P], BF16, kind="ExternalInput")
    whid = nc.dram_tensor("whid", [L, KH, P, KH, P], BF16, kind="ExternalInput")
    wdn = nc.dram_tensor("wdn", [L, KH, P, D], BF16, kind="ExternalInput")
    bup_all = nc.dram_tensor("bup", [L, H], F32, kind="ExternalInput")
    bhid_all = nc.dram_tensor("bhid", [L, H], F32, kind="ExternalInput")
    embT = nc.dram_tensor("embT", [D, VSH], BF16, kind="ExternalInput")
    logits = nc.dram_tensor("logits", [T, VSH], BF16, kind="ExternalOutput")

    with tile.TileContext(nc) as tc, ExitStack() as ctx:
        const = ctx.enter_context(tc.tile_pool(name="const", bufs=1))
        state = ctx.enter_context(tc.tile_pool(name="state", bufs=1))
        actp = ctx.enter_context(tc.tile_pool(name="actp", bufs=1))
        bigp = ctx.enter_context(tc.tile_pool(name="bigp", bufs=1))
        small = ctx.enter_context(tc.tile_pool(name="small", bufs=4))
        stream = ctx.enter_context(tc.tile_pool(name="stream", bufs=2))
        st3 = ctx.enter_context(tc.tile_pool(name="st3", bufs=3))
        ps = ctx.enter_context(tc.tile_pool(name="ps", bufs=1, space="PSUM"))
        dram = ctx.enter_context(tc.tile_pool(name="dram", bufs=2, space="DRAM"))

        identB = const.tile([P, P], BF16)
        make_identity(nc, identB[:])
        eps_t = const.tile([P, 1], F32)
        nc.vector.memset(eps_t, 1e-5)
        ones_t = const.tile([P, 1], BF16)
        nc.vector.memset(ones_t, 1.0)
        mask_sb = const.tile([P, NT, 4, P], F32)
        nc.sync.dma_start(out=mask_sb,
                          in_=maskT.rearrange("j t k q -> k j t q"))

        x_sb = state.tile([P, NT, D], F32)
        nc.sync.dma_start(out=x_sb,
                          in_=x0.rearrange("(tt p) d -> p tt d", p=P))

        def transpose_to(src_block, dst_ap):
            tp = ps.tile([P, P], BF16, name="trps", tag="tr", bufs=2)
            nc.tensor.transpose(out=tp[:], in_=src_block, identity=identB[:])
            nc.any.tensor_copy(out=dst_ap, in_=tp[:])

        def mm_tile(cols=512):
            return ps.tile([P, cols], F32, name="mm", tag="mm", bufs=5,
                           padded_shape=[P, 512])

        def ln_tile(x_ap, out_ap):
            stats = small.tile([P, 2, nc.vector.BN_STATS_DIM], F32,
                               name="lnst", tag="lnst")
            xv = x_ap.rearrange("p (s f) -> p s f", s=2)
            for s in range(2):
                nc.vector.bn_stats(out=stats[:, s, :], in_=xv[:, s, :])
            mv = small.tile([P, nc.vector.BN_AGGR_DIM], F32, name="lnmv",
                            tag="lnmv")
            nc.vector.bn_aggr(out=mv[:], in_=stats[:])
            rstd = small.tile([P, 1], F32, name="lnrs", tag="lnrs")
            nc.scalar.activation(out=rstd[:], in_=mv[:, 1:2], func=AF.Sqrt,
                                 bias=eps_t[:])
            nc.vector.reciprocal(out=rstd[:], in_=rstd[:])
            nc.vector.tensor_scalar(out=out_ap, in0=x_ap,
                                    scalar1=mv[:, 0:1], scalar2=rstd[:],
                                    op0=OP.subtract, op1=OP.mult)

        hT_t, ag_t = {}, {}

        def get_hT(l):
            if l not in hT_t:
                hT_t[l] = actp.tile([P, KD, TPC], BF16, name=f"hT{l}",
                                    tag="hT", bufs=2)
            return hT_t[l]

        def get_ag(l):
            if l not in ag_t:
                agin = dram.tile([NT, P, KD * P], BF16, name=f"agin{l}",
                                 tag="agin")
                agout = dram.tile([NT, 4, P, KD * P], BF16,
                                  name=f"agout{l}", tag="agout")
                ag_t[l] = (agin, agout)
            return ag_t[l]

        def emit_gather_prep(l, ts):
            """LN1 + transpose + hov for token tiles ts of layer l, sharing
            the ov stream across the tiles; fire each tile's AllGather."""
            hT = get_hT(l)
            agin, agout = get_ag(l)
            for t in ts:
                htmp = stream.tile([P, D], BF16, name="htmp", tag="lntmp")
                ln_tile(x_sb[:, t, :], htmp[:])
                for dk in range(KD):
                    transpose_to(htmp[:, dk * P:(dk + 1) * P],
                                 hT[:, dk, t * P:(t + 1) * P])
                nc.sync.dma_start(out=agin[t],
                                  in_=hT[:, :, t * P:(t + 1) * P])
                nc.gpsimd.collective_compute(
                    "AllGather", OP.bypass, replica_groups=GROUPS_BATCH,
                    ins=[agin[t].opt()], outs=[agout[t].opt()])

        def emit_hqT(l, hqT_sb):
            hT = get_hT(l)
            for j in range(KD):
                qkc = stream.tile([P, KD, P], BF16, name="qkc", tag="str2k")
                nc.scalar.dma_start(out=qkc, in_=qkt[l, j])
                mm = mm_tile()
                for k in range(KD):
                    nc.tensor.matmul(out=mm[:], lhsT=qkc[:, k, :],
                                     rhs=hT[:, k, :],
                                     start=(k == 0), stop=(k == KD - 1))
                nc.any.tensor_copy(out=hqT_sb[:, j, :], in_=mm[:])

        def emit_attn_chunk(l, m, hqT_sb, expT_sb, hnat_sb, rs_ps):
            _, agout = get_ag(l)
            qlo = m * P
            hTg = stream.tile([P, 4, KD, P], BF16, name="hTg", tag="hTg",
                              bufs=1)
            nc.sync.dma_start(
                out=hTg,
                in_=agout[m].rearrange("r p (dk n) -> p r dk n", n=P))
            for p in range(4):
                kt = 4 * m + p
                mm = mm_tile(TPC - qlo)
                for dk in range(KD):
                    nc.tensor.matmul(out=mm[:], lhsT=hTg[:, p, dk, :],
                                     rhs=hqT_sb[:, dk, qlo:TPC],
                                     start=(dk == 0), stop=(dk == KD - 1))
                dg = st3.tile([P, P], F32, name="dg", tag="str05")
                nc.vector.tensor_tensor(out=dg[:], in0=mm[:, 0:P],
                                        in1=mask_sb[:, m, p, :], op=OP.add)
                nc.scalar.activation(out=expT_sb[:, kt, qlo:qlo + P],
                                     in_=dg[:], func=AF.Exp, scale=SCALE)
                if m < NT - 1:
                    nc.scalar.activation(
                        out=expT_sb[:, kt, qlo + P:TPC],
                        in_=mm[:, P:], func=AF.Exp, scale=SCALE)
                # reconstruct natural-layout keys for the (attn @ h) matmul
                for dk in range(KD):
                    transpose_to(hTg[:, p, dk, :],
                                 hnat_sb[:, kt, dk * P:(dk + 1) * P])
            nkt = 4 * (m + 1)
            qs = slice(m * P, (m + 1) * P)
            for kt in range(nkt):
                nc.tensor.matmul(out=rs_ps[:, m:m + 1],
                                 lhsT=expT_sb[:, kt, qs], rhs=ones_t[:],
                                 start=(kt == 0), stop=(kt == nkt - 1))
            recip = small.tile([P, 1], F32, name="recip", tag="recip")
            nc.any.tensor_copy(out=recip[:], in_=rs_ps[:, m:m + 1])
            nc.vector.reciprocal(out=recip[:], in_=recip[:])
            # ypreT[d, q] = sum_t h[t, d] * expT[t, q]
            ypre = st3.tile([P, KD, P], BF16, name="ypre", tag="str2k")
            for db in range(KD):
                yp = mm_tile(P)
                for kt in range(nkt):
                    nc.tensor.matmul(
                        out=yp[:], lhsT=hnat_sb[:, kt, db * P:(db + 1) * P],
                        rhs=expT_sb[:, kt, qs],
                        start=(kt == 0), stop=(kt == nkt - 1))
                nc.any.tensor_copy(out=ypre[:, db, :], in_=yp[:])
            # y @ ov, normalize, add into x
            for c in range(2):
                ovb2 = stream.tile([P, KD, 512], BF16, name="ovb2", tag="str8k")
                nc.scalar.dma_start(
                    out=ovb2,
                    in_=ovt[l, :, :, c * 512:(c + 1) * 512]
                    .rearrange("k p n -> p k n"))
                ao = mm_tile()
                for db in range(KD):
                    nc.tensor.matmul(out=ao[:], lhsT=ypre[:, db, :],
                                     rhs=ovb2[:, db, :],
                                     start=(db == 0), stop=(db == KD - 1))
                tmp = st3.tile([P, 512], F32, name="tmp", tag="str2k")
                nc.vector.tensor_scalar_mul(out=tmp[:], in0=ao[:],
                                            scalar1=recip[:])
                nc.vector.tensor_tensor(
                    out=x_sb[:, m, c * 512:(c + 1) * 512],
                    in0=x_sb[:, m, c * 512:(c + 1) * 512],
                    in1=tmp[:], op=OP.add)

        def emit_ln2_up(l, half, mT_sb, m1T_sb, bup_sb):
            ts = (0, 1) if half == 0 else (2, 3)
            off, w = ts[0] * P, 2 * P
            for t in ts:
                mtmp = stream.tile([P, D], BF16, name="mtmp", tag="lntmp")
                ln_tile(x_sb[:, t, :], mtmp[:])
                for dk in range(KD):
                    transpose_to(mtmp[:, dk * P:(dk + 1) * P],
                                 mT_sb[:, dk, t * P:(t + 1) * P])
            for hg in range(KH // 4):
                wt = stream.tile([P, 4, KD, P], BF16, name="wupt", tag="str8k")
                nc.scalar.dma_start(
                    out=wt,
                    in_=wup[l, 4 * hg:4 * hg + 4]
                    .rearrange("h p k n -> p h k n"))
                for hi in range(4):
                    ht = 4 * hg + hi
                    mm = mm_tile(w)
                    for k in range(KD):
                        nc.tensor.matmul(out=mm[:], lhsT=wt[:, hi, k, :],
                                         rhs=mT_sb[:, k, off:off + w],
                                         start=(k == 0), stop=(k == KD - 1))
                    nc.scalar.activation(out=m1T_sb[:, ht, off:off + w],
                                         in_=mm[:], func=AF.Lrelu,
                                         bias=bup_sb[:, ht:ht + 1],
                                         alpha=LRELU)

        def emit_hid(l, half, m1T_sb, m2T_sb, bhid_sb):
            off, w = half * 2 * P, 2 * P
            for ht in range(KH):
                wt = stream.tile([P, KH, P], BF16, name="whidt", tag="str8k")
                nc.scalar.dma_start(out=wt, in_=whid[l, ht])
                mm = mm_tile(w)
                for k in range(KH):
                    nc.tensor.matmul(out=mm[:], lhsT=wt[:, k, :],
                                     rhs=m1T_sb[:, k, off:off + w],
                                     start=(k == 0), stop=(k == KH - 1))
                nc.scalar.activation(out=m2T_sb[:, ht, off:off + w], in_=mm[:],
                                     func=AF.Lrelu, bias=bhid_sb[:, ht:ht + 1],
                                     alpha=LRELU)

        def emit_down(l, half, m2T_sb):
            ts = (0, 1) if half == 0 else (2, 3)
            for c in range(2):
                psl = {t: mm_tile() for t in ts}
                for kg in range(KH // 8):
                    wt = stream.tile([P, 8, 512], BF16, name="wdnt", tag="str8k")
                    nc.scalar.dma_start(
                        out=wt,
                        in_=wdn[l, 8 * kg:8 * kg + 8, :, c * 512:(c + 1) * 512]
                        .rearrange("k p n -> p k n"))
                    for ki in range(8):
                        k = 8 * kg + ki
                        for t in ts:
                            nc.tensor.matmul(
                                out=psl[t][:],
                                lhsT=m2T_sb[:, k, t * P:(t + 1) * P],
                                rhs=wt[:, ki, :], start=(k == 0),
                                stop=(k == KH - 1))
                for t in ts:
                    m3 = st3.tile([P, 512], F32, name="m3ev", tag="str2k")
                    nc.scalar.activation(out=m3[:], in_=psl[t][:], func=AF.Lrelu,
                                         alpha=LRELU)
                    nc.vector.tensor_tensor(
                        out=x_sb[:, t, c * 512:(c + 1) * 512],
                        in0=x_sb[:, t, c * 512:(c + 1) * 512],
                        in1=m3[:], op=OP.add)

        xin = None
        xoutg = {}

        def emit_final_gather(ts):
            nonlocal xin
            if xin is None:
                xin = dram.tile([NT, P, KD * P], BF16, name="xin", tag="xin",
                                bufs=1)
            for t in ts:
                xtmp = stream.tile([P, D], BF16, name="xtmp", tag="lntmp")
                nc.any.tensor_copy(out=xtmp[:], in_=x_sb[:, t, :])
                xT1 = stream.tile([P, KD, P], BF16, name="xT1", tag="str2k")
                for dk in range(KD):
                    transpose_to(xtmp[:, dk * P:(dk + 1) * P], xT1[:, dk, :])
                xo = dram.tile([NCORES, P, KD * P], BF16, name=f"xoutg{t}",
                               tag=f"xoutg{t}", addr_space="Shared", bufs=1)
                xoutg[t] = xo
                nc.sync.dma_start(out=xin[t], in_=xT1[:])
                nc.gpsimd.collective_compute(
                    "AllGather", OP.bypass, replica_groups=GROUP_ALL,
                    ins=[xin[t].opt()], outs=[xo.opt()])

        def emit_unembed():
            emb_lo = bigp.tile([P, KD // 2, VSH], BF16, name="emb_lo",
                               tag="m1T")
            emb_hi = bigp.tile([P, KD // 2, VSH], BF16, name="emb_hi",
                               tag="m2T")
            nc.sync.dma_start(
                out=emb_lo,
                in_=embT[0:D // 2].rearrange("(dk p) v -> p dk v", p=P))
            nc.sync.dma_start(
                out=emb_hi,
                in_=embT[D // 2:D].rearrange("(dk p) v -> p dk v", p=P))

            def embT_tile(dk, vs):
                src = emb_lo if dk < 4 else emb_hi
                return src[:, dk % 4, vs]

            for t in range(NT):
                for r in range(NCORES):
                    xtg = stream.tile([P, KD, P], BF16, name="xtg", tag="str2k")
                    nc.sync.dma_start(
                        out=xtg,
                        in_=xoutg[t][r].rearrange("p (dk n) -> p dk n", n=P))
                    grow = ((r // 4) * S + (4 * t + (r % 4)) * P)
                    lg = stream.tile([P, VSH], BF16, name="lg", tag="str8k")
                    for v in range(NVC):
                        vs = slice(v * VC, (v + 1) * VC)
                        mm = mm_tile(VC)
                        for dk in range(KD):
                            nc.tensor.matmul(out=mm[:], lhsT=xtg[:, dk, :],
                                             rhs=embT_tile(dk, vs),
                                             start=(dk == 0),
                                             stop=(dk == KD - 1))
                        nc.any.tensor_copy(out=lg[:, vs], in_=mm[:])
                    nc.sync.dma_start(out=logits[grow:grow + P, :], in_=lg[:])

        # ---------------- pipeline driver ----------------
        agin0, agout0 = get_ag(0)
        for t in range(NT):
            nc.sync.dma_start(out=agin0[t], in_=h0s[t])
            nc.gpsimd.collective_compute(
                "AllGather", OP.bypass, replica_groups=GROUPS_BATCH,
                ins=[agin0[t].opt()], outs=[agout0[t].opt()])
        hT0 = get_hT(0)
        nc.sync.dma_start(
            out=hT0.rearrange("p dk (t n) -> p dk t n", n=P),
            in_=h0s.rearrange("t p (dk n) -> p dk t n", n=P))
        for l in range(L):
            hqT_sb = actp.tile([P, KD, TPC], BF16, name=f"hqT{l}", tag="hqT")
            expT_sb = bigp.tile([P, 16, TPC], BF16, name=f"expT{l}", tag="expT")
            hnat_sb = bigp.tile([P, 16, D], BF16, name=f"hnat{l}", tag="hovg")
            rs_ps = ps.tile([P, NT], F32, name=f"rs{l}", tag="rs", bufs=1)
            mT_sb = actp.tile([P, KD, TPC], BF16, name=f"mT{l}", tag="hT",
                              bufs=2)
            m1T_sb = bigp.tile([P, KH, TPC], BF16, name=f"m1T{l}", tag="m1T")
            m2T_sb = bigp.tile([P, KH, TPC], BF16, name=f"m2T{l}", tag="m2T")
            bup_sb = small.tile([P, KH], F32, name="bup_sb", tag="bup")
            nc.sync.dma_start(out=bup_sb,
                              in_=bup_all[l].rearrange("(ht p) -> p ht", p=P))
            bhid_sb = small.tile([P, KH], F32, name="bhid_sb", tag="bhid")
            nc.sync.dma_start(out=bhid_sb,
                              in_=bhid_all[l].rearrange("(ht p) -> p ht", p=P))

            emit_hqT(l, hqT_sb)
            emit_attn_chunk(l, 0, hqT_sb, expT_sb, hnat_sb, rs_ps)
            emit_attn_chunk(l, 1, hqT_sb, expT_sb, hnat_sb, rs_ps)
            emit_ln2_up(l, 0, mT_sb, m1T_sb, bup_sb)
            emit_hid(l, 0, m1T_sb, m2T_sb, bhid_sb)
            emit_down(l, 0, m2T_sb)
            if l < L - 1:
                emit_gather_prep(l + 1, (0, 1))
            else:
                emit_final_gather((0, 1))
            emit_attn_chunk(l, 2, hqT_sb, expT_sb, hnat_sb, rs_ps)
            emit_attn_chunk(l, 3, hqT_sb, expT_sb, hnat_sb, rs_ps)
            emit_ln2_up(l, 1, mT_sb, m1T_sb, bup_sb)
            emit_hid(l, 1, m1T_sb, m2T_sb, bhid_sb)
            emit_down(l, 1, m2T_sb)
            if l < L - 1:
                emit_gather_prep(l + 1, (2, 3))
            else:
                emit_final_gather((2, 3))
        emit_unembed()

    nc.compile()
    return nc


_CACHE = {}


def _get_program():
    if "nc" not in _CACHE:
        _CACHE["nc"] = build_program()
    return _CACHE["nc"]


def _bf16(x):
    import ml_dtypes
    return np.ascontiguousarray(x.astype(ml_dtypes.bfloat16))


def _core_token_index(core):
    b, lc = core // 4, core % 4
    idx = []
    for j in range(NT):
        g = 4 * j + lc
        idx.append(b * S + g * P + np.arange(P))
    return np.concatenate(idx)


def _make_maskT(core):
    lc = core % 4
    out = np.zeros((NT, 4, P, P), np.float32)
    k = np.arange(P)[:, None]
    q = np.arange(P)[None, :]
    for j in range(NT):
        for p in range(4):
            if p < lc:
                continue
            elif p == lc:
                out[j, p] = np.where(k <= q, 0.0, MASKV)
            else:
                out[j, p] = MASKV
    return out


def kernel(**inputs):
    tokens = np.asarray(inputs["tokens"]).astype(np.int64).reshape(T)
    emb = np.ascontiguousarray(np.asarray(inputs["emb"], dtype=np.float32))
    qk = np.asarray(inputs["qk"], dtype=np.float32)
    ov = np.asarray(inputs["ov"], dtype=np.float32)
    w_up = np.asarray(inputs["w_up"], dtype=np.float32)
    w_hid = np.asarray(inputs["w_hid"], dtype=np.float32)
    w_down = np.asarray(inputs["w_down"], dtype=np.float32)
    b_up = np.ascontiguousarray(np.asarray(inputs["b_up"], dtype=np.float32))
    b_hid = np.ascontiguousarray(np.asarray(inputs["b_hid"], dtype=np.float32))

    # swizzles: [l, out-block, partition(k%128), k-block, out-within-block]
    qkt = _bf16(qk.reshape(L, KD, P, KD, P).transpose(0, 3, 2, 1, 4))
    ovt = _bf16(ov.reshape(L, KD, P, D))
    wupt = _bf16(w_up.reshape(L, KD, P, KH, P).transpose(0, 3, 2, 1, 4))
    whidt = _bf16(w_hid.reshape(L, KH, P, KH, P).transpose(0, 3, 2, 1, 4))
    wdnt = _bf16(w_down.reshape(L, KH, P, D))

    nc = _get_program()
    in_maps = []
    for core in range(NCORES):
        tidx = _core_token_index(core)
        x0c = np.ascontiguousarray(emb[tokens[tidx]])
        mu = x0c.mean(-1, keepdims=True)
        var = ((x0c - mu) ** 2).mean(-1, keepdims=True)
        h0 = (x0c - mu) / np.sqrt(var + 1e-5)
        h0s = _bf16(h0.reshape(NT, P, KD, P).transpose(0, 3, 2, 1)
                    .reshape(NT, P, KD * P))
        in_maps.append({
            "x0": x0c,
            "h0s": h0s,
            "maskT": _make_maskT(core),
            "qkt": qkt, "ovt": ovt,
            "wup": wupt, "whid": whidt, "wdn": wdnt,
            "bup": b_up, "bhid": b_hid,
            "embT": _bf16(emb[core * VSH:(core + 1) * VSH].T),
        })
    res = run_bass_kernel_spmd(nc, in_maps, core_ids=list(range(NCORES)))
    _CACHE["last"] = res
    shards = [res.results[c]["logits"] for c in range(NCORES)]
    fullb = np.concatenate(shards, axis=1)
    full = np.empty((T, V), np.float32)
    fv = full.view(np.uint32)
    fv[:] = fullb.view(np.uint16).astype(np.uint32) << 16
    return full.reshape(B, S, V)
